# revision 24
# baseline (speedup 1.0000x reference)
"""Grouped-Query Attention kernel for 8 Trainium2 NeuronCores.

Problem: B=2, T=2048, C=2048, H=16 query heads, KV=4 kv heads, D=128.

Exploited reference properties:
  1. RoPE is applied with seq_len = num_heads, so cos/sin depend only on the
     head index (constant over time): RoPE is folded into wq/bq (wk/bk) on
     the host, along with the 1/sqrt(D) score scale.
  2. The "causal mask" is an ADDITIVE +1 on the lower triangle (torch SDPA
     float-mask semantics): exp(S+M) = exp(S)*e^M, so fully-below-diagonal
     score tiles get bias=1.0 inside the exp activation and the 4
     diagonal-crossing tiles get one bf16 elementwise multiply by e^M.

Sharding: core i -> (batch b = i//4, kv-group g = i%4). Each core owns one
KV head and its 4 query heads, computes a partial o_proj over its 512 input
channels; the host sums the 4 partials per batch and adds bo.

Engine budget per core (cost-model): PE ~250us of matmuls is the roofline;
ACT does only the exps (batched [128,2,512] per instruction), DVE does the
softmax-denominator tree + normalize, Pool does PSUM->SBUF output copies.
Phase-3 o_proj matmuls interleave into phase-2 windows with a 5-window lag
so the PE stream never stalls on the softmax chain. All matmul inputs are
bf16 (1 PE cycle/row); accumulation stays f32 in PSUM.
"""

from collections import deque

import numpy as np
import ml_dtypes

import concourse.bass as bass
import concourse.bacc as bacc
import concourse.mybir as mybir
import concourse.tile as tile
from concourse.bass_utils import run_bass_kernel_spmd

F32 = mybir.dt.float32
F32R = mybir.dt.float32r
BF16 = mybir.dt.bfloat16
FP8 = mybir.dt.float8e4
AF = mybir.ActivationFunctionType
DRMODE = mybir.MatmulPerfMode.DoubleRow

# fp8 Dekker-split prescales (power of two; max |v|*scale must stay < 448)
XS = 16.0       # x
WSQ = 8192.0    # rope-folded wq (includes 1/sqrt(D))
WSK = 1024.0    # rope-folded wk
WSV = 1024.0    # wv
WSO = 1024.0    # wo
# (lhsT hl, rhs hl): hi*hi, lo*hi, hi*lo  (lo*lo dropped)
TERMS = ((0, 0), (1, 0), (0, 1))

DIM = 2048
H = 16
KV = 4
D = 128          # head dim
G = H // KV      # 4 query heads per kv head
T = 2048
B = 2
NCORES = 8
ROPE_MAX = 2048
NW = 16          # phase-2 windows: (tch, h)
PO_LAG = 5       # window lag for interleaved o_proj work

_PROGRAM = None


def _build_program():
    # NOTE: the tile framework tracks dependencies per TILE, not per region.
    # Every tensor that has multiple independent producer/consumer slices is
    # split into separate tiles so false WAR/RAW deps don't serialize the
    # pipeline (ps_s pairs, pT quarters, attT per (tch,h), qT per head,
    # v per s-block, weights per ci-group).
    nc = bacc.Bacc(None, target_bir_lowering=False, debug=False)

    # fp8 hi/lo Dekker pairs for phase-1, host-laid-out as
    # [ci-pair P, partition p, 2*hl+ktile, m] so each DMA is 3-dim
    x8_d = nc.declare_dram_parameter("x8", [8, 128, 4, T], FP8, isOutput=False)
    wq8_d = nc.declare_dram_parameter("wq8", [8, 128, 4, 512], FP8,
                                      isOutput=False)
    wk8_d = nc.declare_dram_parameter("wk8", [8, 128, 4, 128], FP8,
                                      isOutput=False)
    wv8_d = nc.declare_dram_parameter("wv8", [8, 128, 4, 128], FP8,
                                      isOutput=False)
    # wo fp8 hi/lo: [head-pair hp, partition p, 2*hl+head-in-pair, oc]
    wo8_d = nc.declare_dram_parameter("wo8", [2, 128, 4, DIM], FP8,
                                      isOutput=False)
    bq_d = nc.declare_dram_parameter("bq", [128, 4], F32, isOutput=False)
    bk_d = nc.declare_dram_parameter("bk", [128, 1], F32, isOutput=False)
    bv_d = nc.declare_dram_parameter("bv_col", [128, 1], F32, isOutput=False)
    ident_d = nc.declare_dram_parameter("ident", [128, 128], BF16, isOutput=False)
    emask_d = nc.declare_dram_parameter("emask", [128, 4, 512], BF16, isOutput=False)
    ones_d = nc.declare_dram_parameter("ones", [128, 128], F32R, isOutput=False)
    o_d = nc.declare_dram_parameter("o_part", [T, DIM], BF16, isOutput=True)

    with tile.TileContext(nc) as tc:
        with tc.tile_pool(name="persist", bufs=1) as persist:
            qT_h = [persist.tile([128, T], BF16, name=f"qT{h}")
                    for h in range(4)]
            kT_c = [persist.tile([128, 512], BF16, name=f"kT{c}")
                    for c in range(4)]                   # [d, s-chunk]
            v_t = [persist.tile([128, 128], BF16, name=f"v{s}")
                   for s in range(16)]                   # [s%128, d]
            emask_sb = persist.tile([128, 4, 512], BF16)
            ones_sb = persist.tile([128, 128], F32R)
            ident_sb = persist.tile([128, 128], BF16)
            wo8_sb = [persist.tile([128, 4, DIM], FP8, name=f"wo8_{hp}")
                      for hp in range(2)]
            # attP_{h,l}[tch][hp]: fp8 hi/lo of 16*normalized attT,
            # [d, head-in-pair, t-chunk]
            attP_h = [[persist.tile([128, 2, 512], FP8, name=f"attH{c}_{p}")
                       for p in range(2)] for c in range(4)]
            attP_l = [[persist.tile([128, 2, 512], FP8, name=f"attL{c}_{p}")
                       for p in range(2)] for c in range(4)]

            # ---------------- phase 1: q/k/v projections ----------------
            with tc.tile_pool(name="ph1w", bufs=1) as ph1w, \
                 tc.tile_pool(name="xtp", bufs=6) as xtp, \
                 tc.tile_pool(name="ps1", bufs=1, space="PSUM") as ps1:
                # per ci-pair P (256 channels): [p, 2*hl+ktile, m]
                wq_p = [ph1w.tile([128, 4, 512], FP8, name=f"wq{P}")
                        for P in range(8)]
                wk_p = [ph1w.tile([128, 4, 128], FP8, name=f"wk{P}")
                        for P in range(8)]
                wv_p = [ph1w.tile([128, 4, 128], FP8, name=f"wv{P}")
                        for P in range(8)]
                vT_sb = ph1w.tile([128, T], BF16)        # [d, s] pre-transpose
                bq_sb = ph1w.tile([128, 4], F32)
                bk_sb = ph1w.tile([128, 1], F32)
                bv_sb = ph1w.tile([128, 1], F32)

                # weight DMAs in consumption order, interleaved with the
                # first t-chunk's x tiles
                xt0 = []
                for P in range(8):
                    nc.sync.dma_start(out=wk_p[P], in_=wk8_d[P])
                    nc.sync.dma_start(out=wv_p[P], in_=wv8_d[P])
                    nc.sync.dma_start(out=wq_p[P], in_=wq8_d[P])
                    xt = xtp.tile([128, 4, 512], FP8, tag="xt", name="xt")
                    nc.sync.dma_start(out=xt, in_=x8_d[P, :, :, 0:512])
                    xt0.append(xt)
                    if P == 2:
                        nc.sync.dma_start(out=bq_sb, in_=bq_d[:])
                        nc.sync.dma_start(out=bk_sb, in_=bk_d[:])
                        nc.sync.dma_start(out=bv_sb, in_=bv_d[:])
                nc.sync.dma_start(out=ident_sb, in_=ident_d[:])

                psqA = ps1.tile([128, 2, 512], F32, tag="psqA")
                psqB = ps1.tile([128, 2, 512], F32, tag="psqB")
                for tch in range(4):
                    if tch == 1:
                        nc.sync.dma_start(out=emask_sb, in_=emask_d[:])
                        nc.sync.dma_start(out=ones_sb, in_=ones_d[:])
                    if tch == 2:
                        nc.sync.dma_start(out=wo8_sb[0], in_=wo8_d[0])
                        nc.sync.dma_start(out=wo8_sb[1], in_=wo8_d[1])
                    tsl = slice(tch * 512, (tch + 1) * 512)
                    psk = ps1.tile([128, 512], F32, tag="psk")
                    psv = ps1.tile([128, 512], F32, tag="psv")
                    for P in range(8):
                        if tch == 0:
                            xt = xt0[P]
                        else:
                            xt = xtp.tile([128, 4, 512], FP8, tag="xt",
                                          name="xt")
                            nc.sync.dma_start(out=xt, in_=x8_d[P, :, :, tsl])
                        for ti, (wh, xh) in enumerate(TERMS):
                            st = dict(start=(P == 0 and ti == 0),
                                      stop=(P == 7 and ti == 2),
                                      perf_mode=DRMODE)
                            xr = xt[:, 2 * xh:2 * xh + 2, :]
                            wkx = wk_p[P][:, 2 * wh:2 * wh + 2, :]
                            wvx = wv_p[P][:, 2 * wh:2 * wh + 2, :]
                            wqx = wq_p[P][:, 2 * wh:2 * wh + 2, :]
                            nc.tensor.matmul(psk, lhsT=wkx, rhs=xr, **st)
                            nc.tensor.matmul(psv, lhsT=wvx, rhs=xr, **st)
                            nc.tensor.matmul(
                                psqA[:, 0, :], lhsT=wqx[:, :, 0:128],
                                rhs=xr, **st)
                            nc.tensor.matmul(
                                psqA[:, 1, :], lhsT=wqx[:, :, 128:256],
                                rhs=xr, **st)
                            nc.tensor.matmul(
                                psqB[:, 0, :], lhsT=wqx[:, :, 256:384],
                                rhs=xr, **st)
                            nc.tensor.matmul(
                                psqB[:, 1, :], lhsT=wqx[:, :, 384:512],
                                rhs=xr, **st)
                        # interleave previous chunk's v transposes
                        if tch > 0 and P < 2:
                            for jj in range(2):
                                si = (tch - 1) * 4 + 2 * P + jj
                                pst = ps1.tile([128, 128], BF16, tag="pst")
                                nc.tensor.transpose(
                                    pst, vT_sb[:, si * 128:(si + 1) * 128],
                                    ident_sb)
                                nc.vector.tensor_copy(v_t[si], pst)
                    sQ = 1.0 / (XS * WSQ)
                    sK = 1.0 / (XS * WSK)
                    sV = 1.0 / (XS * WSV)
                    nc.scalar.activation(
                        kT_c[tch], psk, AF.Identity, bias=bk_sb[:, 0:1],
                        scale=sK)
                    nc.scalar.activation(
                        vT_sb[:, tsl], psv, AF.Identity, bias=bv_sb[:, 0:1],
                        scale=sV)
                    nc.scalar.activation(
                        qT_h[0][:, tsl], psqA[:, 0, :], AF.Identity,
                        bias=bq_sb[:, 0:1], scale=sQ)
                    nc.scalar.activation(
                        qT_h[1][:, tsl], psqA[:, 1, :], AF.Identity,
                        bias=bq_sb[:, 1:2], scale=sQ)
                    nc.scalar.activation(
                        qT_h[2][:, tsl], psqB[:, 0, :], AF.Identity,
                        bias=bq_sb[:, 2:3], scale=sQ)
                    nc.scalar.activation(
                        qT_h[3][:, tsl], psqB[:, 1, :], AF.Identity,
                        bias=bq_sb[:, 3:4], scale=sQ)
                for j in range(4):
                    si = 12 + j
                    pst = ps1.tile([128, 128], BF16, tag="pst")
                    nc.tensor.transpose(
                        pst, vT_sb[:, si * 128:(si + 1) * 128], ident_sb)
                    nc.vector.tensor_copy(v_t[si], pst)

            # ------------- phase 2 (+interleaved o_proj) -------------
            with tc.tile_pool(name="outp", bufs=3) as outp:
                ost_map = {}

                # attP hi/lo x wo8 hi/lo 3-term DoubleRow o_proj:
                # per unit (ttg, oc-pair): 2 oc x [2 head-pairs x 3 terms]
                PO_NJ = 12
                ODESC = 1.0 / (16.0 * WSO)

                def po_step(pool, j, u, po_state):
                    ttg = u // 2
                    tc_src, tt = ttg // 4, ttg % 4
                    oc = (u % 2) * 2 + j // 6
                    step = j % 6
                    hp = step // 3
                    wh, ah = TERMS[step % 3]
                    if step == 0:
                        po_state[0] = pool.tile(
                            [128, 512], F32, tag="po", name="po")
                    if j == 0 and u % 2 == 0:
                        ost_map[ttg] = outp.tile(
                            [128, DIM], BF16, tag="ostage", name="ostage")
                    po = po_state[0]
                    att = (attP_h if ah == 0 else attP_l)[tc_src][hp]
                    nc.tensor.matmul(
                        po,
                        lhsT=att[:, :, tt * 128:(tt + 1) * 128],
                        rhs=wo8_sb[hp][:, 2 * wh:2 * wh + 2,
                                       oc * 512:(oc + 1) * 512],
                        start=(step == 0), stop=(step == 5),
                        perf_mode=DRMODE)
                    if step == 5:
                        nc.vector.tensor_scalar_mul(
                            ost_map[ttg][:, oc * 512:(oc + 1) * 512], po,
                            ODESC)
                    if step == 5 and u == 2 * NW - 1:
                        if oc == 2:
                            nc.sync.dma_start(
                                out=o_d[ttg * 128:(ttg + 1) * 128, 0:1536],
                                in_=ost_map[ttg][:, 0:1536])
                        else:
                            nc.sync.dma_start(
                                out=o_d[ttg * 128:(ttg + 1) * 128, 1536:2048],
                                in_=ost_map[ttg][:, 1536:2048])
                            ost_map.pop(ttg)
                    elif j == PO_NJ - 1 and u % 2 == 1:
                        nc.sync.dma_start(
                            out=o_d[ttg * 128:(ttg + 1) * 128, :],
                            in_=ost_map.pop(ttg))

                with tc.tile_pool(name="pTp", bufs=2) as pTp, \
                     tc.tile_pool(name="trp", bufs=1) as trp, \
                     tc.tile_pool(name="smp", bufs=2) as smp, \
                     tc.tile_pool(name="ps2s", bufs=1, space="PSUM") as ps2s, \
                     tc.tile_pool(name="psa", bufs=2, space="PSUM") as psa, \
                     tc.tile_pool(name="pop", bufs=2, space="PSUM") as pop:
                    # two independent score-pair tiles (even/odd pairs) so
                    # the score->exp WAR distance is truly 2 pairs
                    psAB = [ps2s.tile([128, 2, 512], F32, tag=f"ps{i}",
                                      name=f"ps{i}") for i in range(2)]
                    po_state = [None]
                    prev = None
                    carry = None

                    def emit_denom_flush(pv):
                        # ones_sb holds 1/16 -> rinv = 16/D; tmp = 16*attT;
                        # fp8 Dekker hi (DVE) + lo (Pool) for the o_proj DR
                        accv_p, ps_att_p, tch_p, h_p = pv
                        ps_sum = pop.tile([128, 512], F32, tag="po",
                                          name="pss")
                        nc.tensor.matmul(
                            ps_sum, lhsT=ones_sb, rhs=accv_p,
                            start=True, stop=True)
                        rinv = smp.tile([128, 512], F32, tag="rinv",
                                        name="rinv")
                        nc.vector.reciprocal(rinv, ps_sum)
                        tmp = smp.tile([128, 512], BF16, tag="tmp",
                                       name="tmp")
                        nc.vector.tensor_mul(tmp, ps_att_p, rinv)
                        hp, sl = h_p // 2, h_p % 2
                        hi = attP_h[tch_p][hp]
                        nc.vector.tensor_copy(hi[:, sl, :], tmp)
                        nc.gpsimd.tensor_sub(
                            attP_l[tch_p][hp][:, sl, :], tmp, hi[:, sl, :])

                    for w in range(NW):
                        tch, h = w // 4, w % 4
                        tsl = slice(tch * 512, (tch + 1) * 512)
                        diag0 = 4 * tch
                        # pT quarters: pTq[qi] holds exp tiles 4qi..4qi+3
                        pTq = [pTp.tile([128, 4, 512], BF16, tag=f"pT{qi}",
                                        name=f"pT{qi}") for qi in range(4)]
                        s8 = trp.tile([128, 8, 512], BF16, tag="s8", name="s8")
                        l4 = trp.tile([128, 4, 512], BF16, tag="l4", name="l4")
                        l2 = trp.tile([128, 2, 512], BF16, tag="l2", name="l2")
                        accv = trp.tile([128, 512], F32R, tag="accv",
                                        name="accv")
                        ps_att = psa.tile([128, 512], F32, tag="pa", name="pa")
                        u = 2 * (w - PO_LAG) if w >= PO_LAG else -1

                        pending = deque(
                            [a for a in range(16)
                             if not diag0 <= a < diag0 + 4]
                            + list(range(diag0, diag0 + 4)))
                        att_started = [False]

                        def emit_att(a, is_last, ps_att=ps_att, pTq=pTq,
                                     att_started=att_started):
                            nc.tensor.matmul(
                                ps_att, lhsT=v_t[a],
                                rhs=pTq[a // 4][:, a % 4, :],
                                start=(not att_started[0]), stop=is_last)
                            att_started[0] = True

                        def drain_atts(si_last, limit=99):
                            n = 0
                            while pending and n < limit:
                                a = pending[0]
                                if (a | 1) > si_last - 1:
                                    break
                                if (diag0 <= a < diag0 + 4
                                        and si_last < diag0 + 3):
                                    break
                                pending.popleft()
                                emit_att(a, False)
                                n += 1

                        def emit_score(si):
                            nc.tensor.matmul(
                                psAB[(si // 2) % 2][:, si % 2, :],
                                lhsT=kT_c[si // 4][
                                    :, (si % 4) * 128:(si % 4 + 1) * 128],
                                rhs=qT_h[h][:, tsl],
                                start=True, stop=True)
                            if si % 2 == 1:
                                p2 = si // 2
                                k = si // 4
                                qi = k
                                bias = 1.0 if k < tch else 0.0
                                nc.scalar.activation(
                                    pTq[qi][:, 2 * (p2 % 2):2 * (p2 % 2) + 2, :],
                                    psAB[p2 % 2], AF.Exp, bias=bias)
                                if si == diag0 + 3:
                                    nc.vector.tensor_mul(
                                        pTq[tch], pTq[tch], emask_sb)
                                    nc.vector.tensor_add(
                                        s8[:, p2 - 1, :],
                                        pTq[qi][:, 0, :], pTq[qi][:, 1, :])
                                    nc.vector.tensor_add(
                                        s8[:, p2, :],
                                        pTq[qi][:, 2, :], pTq[qi][:, 3, :])
                                elif si != diag0 + 1:
                                    nc.vector.tensor_add(
                                        s8[:, p2, :],
                                        pTq[qi][:, 2 * (p2 % 2), :],
                                        pTq[qi][:, 2 * (p2 % 2) + 1, :])
                                if si % 4 == 3:
                                    nc.vector.tensor_add(
                                        l4[:, k, :],
                                        s8[:, 2 * k, :], s8[:, 2 * k + 1, :])
                                if si % 8 == 7:
                                    q8 = si // 8
                                    nc.vector.tensor_add(
                                        l2[:, q8, :],
                                        l4[:, 2 * q8, :], l4[:, 2 * q8 + 1, :])
                                if si == 15:
                                    nc.vector.tensor_add(
                                        accv, l2[:, 0, :], l2[:, 1, :])

                        # 4 bursts; scores mid-slot so exps start early;
                        # atts lag one burst; o_proj 6 DR matmuls/burst
                        for k in range(4):
                            uk = u + k // 2 if u >= 0 else -1
                            jk = (6 * k) % PO_NJ
                            if uk >= 0:
                                po_step(pop, jk, uk, po_state)
                                po_step(pop, jk + 1, uk, po_state)
                            drain_atts(4 * k, limit=2)
                            emit_score(4 * k)
                            emit_score(4 * k + 1)
                            if k == 0:
                                if carry is not None:
                                    cpending, cemit = carry
                                    while cpending:
                                        a = cpending.popleft()
                                        cemit(a, not cpending)
                                    carry = None
                                if prev is not None:
                                    emit_denom_flush(prev)
                                    prev = None
                            if uk >= 0:
                                po_step(pop, jk + 2, uk, po_state)
                                po_step(pop, jk + 3, uk, po_state)
                            drain_atts(4 * k, limit=2)
                            emit_score(4 * k + 2)
                            emit_score(4 * k + 3)
                            if uk >= 0:
                                po_step(pop, jk + 4, uk, po_state)
                                po_step(pop, jk + 5, uk, po_state)
                            drain_atts(4 * k)
                        carry = (pending, emit_att)
                        prev = (accv, ps_att, tch, h)

                    # last window's att drain + softmax flush
                    cpending, cemit = carry
                    while cpending:
                        a = cpending.popleft()
                        cemit(a, not cpending)
                    emit_denom_flush(prev)

                # ------------- phase 3 tail: remaining o_proj units ----------
                with tc.tile_pool(name="pot", bufs=4, space="PSUM") as pot:
                    po_state = [None]
                    for u in range(2 * (NW - PO_LAG), 2 * NW):
                        for j in range(PO_NJ):
                            po_step(pot, j, u, po_state)
    nc.finalize()
    return nc


def _get_program():
    global _PROGRAM
    if _PROGRAM is None:
        _PROGRAM = _build_program()
    return _PROGRAM


def _rope_cos_sin():
    inv_freq = 1.0 / (10000.0 ** (np.arange(0, D, 2, dtype=np.float64) / D))
    t = np.arange(ROPE_MAX, dtype=np.float64)
    freqs = np.outer(t, inv_freq)                       # [S, D/2]
    emb = np.concatenate([freqs, freqs], axis=-1)       # [S, D]
    return np.cos(emb).astype(np.float32), np.sin(emb).astype(np.float32)


def _fold_rope(w, b, nheads, scale):
    """Fold per-head RoPE (position index = head index) into weight rows.

    w: [nheads*D, C], b: [nheads*D]. Returns rotated (and scaled) copies.
    rope(q)[i] = q[i]*cos[i] + rot_half(q)[i]*sin[i],
    rot_half(q)[i] = -q[i+64] (i<64) else q[i-64].
    """
    cos, sin = _rope_cos_sin()
    w = w.reshape(nheads, D, -1)
    b = b.reshape(nheads, D)
    c = cos[:nheads][:, :, None]          # [nheads, D, 1]
    s = sin[:nheads][:, :, None]
    w_rot = np.empty_like(w)
    hD = D // 2
    w_rot[:, :hD] = w[:, :hD] * c[:, :hD] - w[:, hD:] * s[:, :hD]
    w_rot[:, hD:] = w[:, hD:] * c[:, hD:] + w[:, :hD] * s[:, hD:]
    cb = cos[:nheads]
    sb = sin[:nheads]
    b_rot = np.empty_like(b)
    b_rot[:, :hD] = b[:, :hD] * cb[:, :hD] - b[:, hD:] * sb[:, :hD]
    b_rot[:, hD:] = b[:, hD:] * cb[:, hD:] + b[:, :hD] * sb[:, hD:]
    return (w_rot.reshape(nheads * D, -1) * scale).astype(np.float32), \
           (b_rot.reshape(nheads * D) * scale).astype(np.float32)


BF = ml_dtypes.bfloat16
F8 = ml_dtypes.float8_e4m3fn


def _split8(a, prescale):
    """Dekker split of (a*prescale) into fp8 hi+lo, laid out as
    [ci-pair P, partition p, 2*hl + ktile, m] for 3-dim pair DMAs.

    a: [DIM, m] (contraction-major)."""
    s = (a.astype(np.float32) * np.float32(prescale))
    hi = s.astype(F8)
    lo = (s - hi.astype(np.float32)).astype(F8)
    hl = np.stack([hi, lo], axis=0)          # [2, DIM, m]
    m = a.shape[1]
    hl = hl.reshape(2, 8, 2, 128, m)         # [hl, P, i, p, m]
    hl = hl.transpose(1, 3, 0, 2, 4)         # [P, p, hl, i, m]
    return np.ascontiguousarray(hl.reshape(8, 128, 4, m))


def _host_inputs(x, wq, bq, wk, bk, wv, bv, wo, bo):
    """Build the per-core input maps."""
    scale = float(D) ** -0.5
    wq_r, bq_r = _fold_rope(wq.astype(np.float32), bq.astype(np.float32), H, scale)
    wk_r, bk_r = _fold_rope(wk.astype(np.float32), bk.astype(np.float32), KV, 1.0)

    # diagonal tile multiplier: e^mask, mask[p, j, t'] = 1 if j*128 + p <= t'
    p_idx = np.arange(128)[:, None, None]
    j_idx = np.arange(4)[None, :, None]
    t_idx = np.arange(512)[None, None, :]
    emask = np.where((j_idx * 128 + p_idx) <= t_idx,
                     np.float32(np.e), np.float32(1.0)).astype(BF)

    x8 = [_split8(x[b].T, XS) for b in range(B)]
    wq8 = {}
    wk8 = {}
    wv8 = {}
    wo8 = {}
    for g in range(G):
        qs = slice(512 * g, 512 * (g + 1))
        ks = slice(128 * g, 128 * (g + 1))
        wq8[g] = _split8(wq_r[qs].T, WSQ)
        wk8[g] = _split8(wk_r[ks].T, WSK)
        wv8[g] = _split8(wv[ks].astype(np.float32).T, WSV)
        # woT [512 c, 2048 oc] -> [hp, p, 2*hl+i, oc], c = (2*hp+i)*128+p
        woT = wo[:, qs].astype(np.float32).T
        s = woT * np.float32(WSO)
        hi = s.astype(F8)
        lo = (s - hi.astype(np.float32)).astype(F8)
        hl = np.stack([hi, lo], axis=0)          # [hl, 512, oc]
        hl = hl.reshape(2, 2, 2, 128, DIM)       # [hl, hp, i, p, oc]
        hl = hl.transpose(1, 3, 0, 2, 4)         # [hp, p, hl, i, oc]
        wo8[g] = np.ascontiguousarray(hl.reshape(2, 128, 4, DIM))

    in_maps = []
    for core in range(NCORES):
        b, g = divmod(core, G)
        qs = slice(512 * g, 512 * (g + 1))
        ks = slice(128 * g, 128 * (g + 1))
        in_maps.append({
            "x8": x8[b],
            "wq8": wq8[g],
            "wk8": wk8[g],
            "wv8": wv8[g],
            "wo8": wo8[g],
            "bq": np.ascontiguousarray(bq_r[qs].reshape(4, 128).T),
            "bk": np.ascontiguousarray(bk_r[ks].reshape(128, 1)),
            "bv_col": np.ascontiguousarray(
                bv[ks].astype(np.float32).reshape(128, 1)),
            "ident": np.eye(128, dtype=np.float32).astype(BF),
            "emask": emask,
            # 1/16: ps_sum = D/16 so rinv = 16/D (attP = fp8(16*attT_norm))
            "ones": np.full((128, 128), 1.0 / 16.0, dtype=np.float32),
        })
    return in_maps


def run_cores(inputs, trace=False, **kw):
    nc = _get_program()
    in_maps = _host_inputs(**inputs)
    res = run_bass_kernel_spmd(nc, in_maps, list(range(NCORES)), trace=trace, **kw)
    return res


def assemble(results, bo):
    bo = bo.astype(np.float32)
    out = np.empty((B, T, DIM), dtype=np.float32)
    for b in range(B):
        acc = results[b * G + 0]["o_part"].astype(np.float32)
        for g in range(1, G):
            acc = acc + results[b * G + g]["o_part"].astype(np.float32)
        out[b] = acc + bo
    return out


def kernel(**inputs) -> np.ndarray:
    res = run_cores(inputs)
    return assemble(res.results, inputs["bo"])



# revision 64
# speedup vs baseline: 1.0340x; 1.0340x over previous
"""Grouped-Query Attention kernel for 8 Trainium2 NeuronCores.

Problem: B=2, T=2048, C=2048, H=16 query heads, KV=4 kv heads, D=128.

Exploited reference properties:
  1. RoPE is applied with seq_len = num_heads, so cos/sin depend only on the
     head index (constant over time): RoPE is folded into wq/bq (wk/bk) on
     the host, along with the 1/sqrt(D) score scale.
  2. The "causal mask" is an ADDITIVE +1 on the lower triangle (torch SDPA
     float-mask semantics): exp(S+M) = exp(S)*e^M, so fully-below-diagonal
     score tiles get bias=1.0 inside the exp activation and the 4
     diagonal-crossing tiles get one bf16 elementwise multiply by e^M.

Sharding: core i -> (batch b = i//4, kv-group g = i%4). Each core owns one
KV head and its 4 query heads, computes a partial o_proj over its 512 input
channels; the host sums the 4 partials per batch and adds bo.

Engine budget per core (cost-model): PE ~250us of matmuls is the roofline;
ACT does only the exps (batched [128,2,512] per instruction), DVE does the
softmax-denominator tree + normalize, Pool does PSUM->SBUF output copies.
Phase-3 o_proj matmuls interleave into phase-2 windows with a 5-window lag
so the PE stream never stalls on the softmax chain. All matmul inputs are
bf16 (1 PE cycle/row); accumulation stays f32 in PSUM.
"""

from collections import deque

import numpy as np
import ml_dtypes

import concourse.bass as bass
import concourse.bacc as bacc
import concourse.mybir as mybir
import concourse.tile as tile
from concourse.bass_utils import run_bass_kernel_spmd

F32 = mybir.dt.float32
F32R = mybir.dt.float32r
BF16 = mybir.dt.bfloat16
FP8 = mybir.dt.float8e4
AF = mybir.ActivationFunctionType
DRMODE = mybir.MatmulPerfMode.DoubleRow

# fp8 Dekker-split prescales (power of two; max |v|*scale must stay < 448)
XS = 16.0       # x
WSQ = 8192.0    # rope-folded wq (includes 1/sqrt(D))
WSK = 1024.0    # rope-folded wk
WSV = 1024.0    # wv
WSO = 1024.0    # wo
# (lhsT hl, rhs hl): hi*hi, lo*hi, hi*lo  (lo*lo dropped)
TERMS = ((0, 0), (1, 0), (0, 1))

DIM = 2048
H = 16
KV = 4
D = 128          # head dim
G = H // KV      # 4 query heads per kv head
T = 2048
B = 2
NCORES = 8
ROPE_MAX = 2048
NW = 16          # phase-2 windows: (tch, h)
PO_LAG = 5       # window lag for interleaved o_proj work

_PROGRAM = None


def _build_program():
    # NOTE: the tile framework tracks dependencies per TILE, not per region.
    # Every tensor that has multiple independent producer/consumer slices is
    # split into separate tiles so false WAR/RAW deps don't serialize the
    # pipeline (ps_s pairs, pT quarters, attT per (tch,h), qT per head,
    # v per s-block, weights per ci-group).
    nc = bacc.Bacc(None, target_bir_lowering=False, debug=False)

    # fp8 hi/lo Dekker pairs for phase-1, host-laid-out as
    # [partition p, 4*P + 2*hl + ktile, m] so whole-tensor DMAs are 3-dim
    x8_d = nc.declare_dram_parameter("x8", [128, 32, T], FP8, isOutput=False)
    wq8_d = nc.declare_dram_parameter("wq8", [128, 32, 512], FP8,
                                      isOutput=False)
    wk8_d = nc.declare_dram_parameter("wk8", [128, 32, 128], FP8,
                                      isOutput=False)
    wv8_d = nc.declare_dram_parameter("wv8", [128, 32, 128], FP8,
                                      isOutput=False)
    # wo fp8 hi/lo: [head-pair hp, partition p, 2*hl+head-in-pair, oc]
    wo8_d = nc.declare_dram_parameter("wo8", [2, 128, 4, DIM], FP8,
                                      isOutput=False)
    bq_d = nc.declare_dram_parameter("bq", [128, 4], F32, isOutput=False)
    bk_d = nc.declare_dram_parameter("bk", [128, 1], F32, isOutput=False)
    bv_d = nc.declare_dram_parameter("bv_col", [128, 1], F32, isOutput=False)
    ident_d = nc.declare_dram_parameter("ident", [128, 128], BF16, isOutput=False)
    emask_d = nc.declare_dram_parameter("emask", [128, 4, 512], BF16, isOutput=False)
    ones_d = nc.declare_dram_parameter("ones", [128, 128], F32R, isOutput=False)
    o_d = nc.declare_dram_parameter("o_part", [T, DIM], BF16, isOutput=True)

    with tile.TileContext(nc) as tc:
        with tc.tile_pool(name="persist", bufs=1) as persist:
            qT_h = [persist.tile([128, T], BF16, name=f"qT{h}")
                    for h in range(4)]
            kT_c = [persist.tile([128, 512], BF16, name=f"kT{c}")
                    for c in range(4)]                   # [d, s-chunk]
            v_t = [persist.tile([128, 128], BF16, name=f"v{s}")
                   for s in range(16)]                   # [s%128, d]
            emask_sb = persist.tile([128, 4, 512], BF16)
            ones_sb = persist.tile([128, 128], F32R)
            ident_sb = persist.tile([128, 128], BF16)
            wo8_sb = [persist.tile([128, 4, DIM], FP8, name=f"wo8_{hp}")
                      for hp in range(2)]
            # attP_{h,l}[tch][hp]: fp8 hi/lo of 16*normalized attT,
            # [d, head-in-pair, t-chunk]
            attP_h = [[persist.tile([128, 2, 512], FP8, name=f"attH{c}_{p}")
                       for p in range(2)] for c in range(4)]
            attP_l = [[persist.tile([128, 2, 512], FP8, name=f"attL{c}_{p}")
                       for p in range(2)] for c in range(4)]

            # ---------------- phase 1: q/k/v projections ----------------
            with tc.tile_pool(name="ph1w", bufs=1) as ph1w, \
                 tc.tile_pool(name="xtp", bufs=10) as xtp, \
                 tc.tile_pool(name="ps1", bufs=1, space="PSUM") as ps1:
                # [p, 4*P + 2*hl + ktile, m]; 2-pair DMA granularity keeps
                # the HWDGE fixed cost per DMA amortized
                wq_p = [ph1w.tile([128, 4, 512], FP8, name=f"wq{P}")
                        for P in range(8)]
                wk_j = [ph1w.tile([128, 8, 128], FP8, name=f"wk{J}")
                        for J in range(4)]
                wv_j = [ph1w.tile([128, 8, 128], FP8, name=f"wv{J}")
                        for J in range(4)]
                vT_sb = ph1w.tile([128, T], BF16)        # [d, s] pre-transpose
                bq_sb = ph1w.tile([128, 4], F32)
                bk_sb = ph1w.tile([128, 1], F32)
                bv_sb = ph1w.tile([128, 1], F32)

                # weight DMAs in consumption order, interleaved with the
                # first t-chunk's x tiles
                xt0 = []
                # weights stream on the SP HWDGE queue in consumption
                # order; x tiles (2 pairs each) on the ACT HWDGE queue
                for J in range(4):
                    xt = xtp.tile([128, 8, 512], FP8, tag="xt", name="xt")
                    nc.scalar.dma_start(
                        out=xt, in_=x8_d[:, 8 * J:8 * (J + 1), 0:512])
                    xt0.append(xt)
                    nc.sync.dma_start(
                        out=wk_j[J], in_=wk8_d[:, 8 * J:8 * (J + 1), :])
                    nc.sync.dma_start(
                        out=wv_j[J], in_=wv8_d[:, 8 * J:8 * (J + 1), :])
                    for P in (2 * J, 2 * J + 1):
                        nc.sync.dma_start(
                            out=wq_p[P],
                            in_=wq8_d[:, 4 * P:4 * (P + 1), :])
                    if J == 1:
                        nc.sync.dma_start(out=bq_sb, in_=bq_d[:])
                        nc.sync.dma_start(out=bk_sb, in_=bk_d[:])
                        nc.sync.dma_start(out=bv_sb, in_=bv_d[:])
                nc.sync.dma_start(out=ident_sb, in_=ident_d[:])

                psqA = ps1.tile([128, 2, 512], F32, tag="psqA")
                psqB = ps1.tile([128, 2, 512], F32, tag="psqB")
                for tch in range(4):
                    if tch == 1:
                        nc.sync.dma_start(out=emask_sb, in_=emask_d[:])
                        nc.sync.dma_start(out=ones_sb, in_=ones_d[:])
                    tsl = slice(tch * 512, (tch + 1) * 512)
                    psk = ps1.tile([128, 512], F32, tag="psk")
                    psv = ps1.tile([128, 512], F32, tag="psv")
                    for P in range(8):
                        J, lp = P // 2, (P % 2) * 4
                        if tch == 0:
                            xt = xt0[J]
                        else:
                            if P % 2 == 0:
                                xt = xtp.tile([128, 8, 512], FP8, tag="xt",
                                              name="xt")
                                nc.scalar.dma_start(
                                    out=xt,
                                    in_=x8_d[:, 8 * J:8 * (J + 1), tsl])
                                xt_cur = xt
                            xt = xt_cur
                        for ti, (wh, xh) in enumerate(TERMS):
                            st = dict(start=(P == 0 and ti == 0),
                                      stop=(P == 7 and ti == 2),
                                      perf_mode=DRMODE)
                            xsl = slice(lp + 2 * xh, lp + 2 * xh + 2)
                            wsl = slice(lp + 2 * wh, lp + 2 * wh + 2)
                            xr = xt[:, xsl, :]
                            wkx = wk_j[J][:, wsl, :]
                            wvx = wv_j[J][:, wsl, :]
                            wqx = wq_p[P][:, 2 * wh:2 * wh + 2, :]
                            nc.tensor.matmul(psk, lhsT=wkx, rhs=xr, **st)
                            nc.tensor.matmul(psv, lhsT=wvx, rhs=xr, **st)
                            nc.tensor.matmul(
                                psqA[:, 0, :], lhsT=wqx[:, :, 0:128],
                                rhs=xr, **st)
                            nc.tensor.matmul(
                                psqA[:, 1, :], lhsT=wqx[:, :, 128:256],
                                rhs=xr, **st)
                            nc.tensor.matmul(
                                psqB[:, 0, :], lhsT=wqx[:, :, 256:384],
                                rhs=xr, **st)
                            nc.tensor.matmul(
                                psqB[:, 1, :], lhsT=wqx[:, :, 384:512],
                                rhs=xr, **st)
                        # interleave previous chunk's v transposes
                        if tch > 0 and P < 2:
                            for jj in range(2):
                                si = (tch - 1) * 4 + 2 * P + jj
                                pst = ps1.tile([128, 128], BF16, tag="pst")
                                nc.tensor.transpose(
                                    pst, vT_sb[:, si * 128:(si + 1) * 128],
                                    ident_sb)
                                nc.vector.tensor_copy(v_t[si], pst)
                    sQ = 1.0 / (XS * WSQ)
                    sK = 1.0 / (XS * WSK)
                    sV = 1.0 / (XS * WSV)
                    nc.scalar.activation(
                        kT_c[tch], psk, AF.Identity, bias=bk_sb[:, 0:1],
                        scale=sK)
                    nc.scalar.activation(
                        vT_sb[:, tsl], psv, AF.Identity, bias=bv_sb[:, 0:1],
                        scale=sV)
                    nc.scalar.activation(
                        qT_h[0][:, tsl], psqA[:, 0, :], AF.Identity,
                        bias=bq_sb[:, 0:1], scale=sQ)
                    nc.scalar.activation(
                        qT_h[1][:, tsl], psqA[:, 1, :], AF.Identity,
                        bias=bq_sb[:, 1:2], scale=sQ)
                    nc.scalar.activation(
                        qT_h[2][:, tsl], psqB[:, 0, :], AF.Identity,
                        bias=bq_sb[:, 2:3], scale=sQ)
                    nc.scalar.activation(
                        qT_h[3][:, tsl], psqB[:, 1, :], AF.Identity,
                        bias=bq_sb[:, 3:4], scale=sQ)
                for j in range(4):
                    si = 12 + j
                    pst = ps1.tile([128, 128], BF16, tag="pst")
                    nc.tensor.transpose(
                        pst, vT_sb[:, si * 128:(si + 1) * 128], ident_sb)
                    nc.vector.tensor_copy(v_t[si], pst)

            # ------------- phase 2 (+interleaved o_proj) -------------
            with tc.tile_pool(name="outp", bufs=3) as outp:
                ost_map = {}

                # attP hi/lo x wo8 hi/lo 3-term DoubleRow o_proj:
                # per unit (ttg, oc-pair): 2 oc x [2 head-pairs x 3 terms]
                PO_NJ = 12
                ODESC = 1.0 / (16.0 * WSO)

                def po_step(pool, j, u, po_state):
                    ttg = u // 2
                    tc_src, tt = ttg // 4, ttg % 4
                    oc = (u % 2) * 2 + j // 6
                    step = j % 6
                    hp = step // 3
                    wh, ah = TERMS[step % 3]
                    if step == 0:
                        po_state[0] = pool.tile(
                            [128, 512], F32, tag="po", name="po")
                    if j == 0 and u % 2 == 0:
                        ost_map[ttg] = outp.tile(
                            [128, DIM], BF16, tag="ostage", name="ostage")
                    po = po_state[0]
                    att = (attP_h if ah == 0 else attP_l)[tc_src][hp]
                    nc.tensor.matmul(
                        po,
                        lhsT=att[:, :, tt * 128:(tt + 1) * 128],
                        rhs=wo8_sb[hp][:, 2 * wh:2 * wh + 2,
                                       oc * 512:(oc + 1) * 512],
                        start=(step == 0), stop=(step == 5),
                        perf_mode=DRMODE)
                    if step == 5:
                        nc.vector.tensor_scalar_mul(
                            ost_map[ttg][:, oc * 512:(oc + 1) * 512], po,
                            ODESC)
                    if step == 5 and u == 2 * NW - 1:
                        if oc == 2:
                            nc.sync.dma_start(
                                out=o_d[ttg * 128:(ttg + 1) * 128, 0:1536],
                                in_=ost_map[ttg][:, 0:1536])
                        else:
                            nc.sync.dma_start(
                                out=o_d[ttg * 128:(ttg + 1) * 128, 1536:2048],
                                in_=ost_map[ttg][:, 1536:2048])
                            ost_map.pop(ttg)
                    elif j == PO_NJ - 1 and u % 2 == 1:
                        nc.sync.dma_start(
                            out=o_d[ttg * 128:(ttg + 1) * 128, :],
                            in_=ost_map.pop(ttg))

                with tc.tile_pool(name="pTp", bufs=2) as pTp, \
                     tc.tile_pool(name="trp", bufs=1) as trp, \
                     tc.tile_pool(name="smp", bufs=2) as smp, \
                     tc.tile_pool(name="ps2s", bufs=1, space="PSUM") as ps2s, \
                     tc.tile_pool(name="psa", bufs=2, space="PSUM") as psa, \
                     tc.tile_pool(name="pop", bufs=2, space="PSUM") as pop:
                    # two independent score-pair tiles (even/odd pairs) so
                    # the score->exp WAR distance is truly 2 pairs
                    psAB = [ps2s.tile([128, 2, 512], F32, tag=f"ps{i}",
                                      name=f"ps{i}") for i in range(2)]
                    po_state = [None]
                    prev = None
                    carry = None

                    def emit_denom_flush(pv):
                        # ones_sb holds 1/16 -> rinv = 16/D; tmp = 16*attT;
                        # fp8 Dekker hi (DVE) + lo (Pool) for the o_proj DR
                        accv_p, ps_att_p, tch_p, h_p = pv
                        ps_sum = pop.tile([128, 512], F32, tag="po",
                                          name="pss")
                        nc.tensor.matmul(
                            ps_sum, lhsT=ones_sb, rhs=accv_p,
                            start=True, stop=True)
                        rinv = smp.tile([128, 512], F32, tag="rinv",
                                        name="rinv")
                        nc.vector.reciprocal(rinv, ps_sum)
                        tmp = smp.tile([128, 512], BF16, tag="tmp",
                                       name="tmp")
                        nc.vector.tensor_mul(tmp, ps_att_p, rinv)
                        hp, sl = h_p // 2, h_p % 2
                        hi = attP_h[tch_p][hp]
                        # last t-chunk: Pool's queue backs up the tail po
                        # units, so split on DVE there instead
                        eng = nc.vector if tch_p == 3 else nc.gpsimd
                        eng.tensor_copy(hi[:, sl, :], tmp)
                        eng.tensor_sub(
                            attP_l[tch_p][hp][:, sl, :], tmp, hi[:, sl, :])

                    for w in range(NW):
                        if w == 0:
                            # wo8 is first needed at window PO_LAG; load it
                            # over the idle phase-2 SP queue
                            nc.sync.dma_start(out=wo8_sb[0], in_=wo8_d[0])
                            nc.sync.dma_start(out=wo8_sb[1], in_=wo8_d[1])
                        tch, h = w // 4, w % 4
                        tsl = slice(tch * 512, (tch + 1) * 512)
                        diag0 = 4 * tch
                        # pT quarters: pTq[qi] holds exp tiles 4qi..4qi+3
                        pTq = [pTp.tile([128, 4, 512], BF16, tag=f"pT{qi}",
                                        name=f"pT{qi}") for qi in range(4)]
                        s8 = trp.tile([128, 8, 512], BF16, tag="s8", name="s8")
                        l4 = trp.tile([128, 4, 512], BF16, tag="l4", name="l4")
                        l2 = trp.tile([128, 2, 512], BF16, tag="l2", name="l2")
                        accv = trp.tile([128, 512], F32R, tag="accv",
                                        name="accv")
                        ps_att = psa.tile([128, 512], F32, tag="pa", name="pa")
                        u = 2 * (w - PO_LAG) if w >= PO_LAG else -1

                        pending = deque(
                            [a for a in range(16)
                             if not diag0 <= a < diag0 + 4]
                            + list(range(diag0, diag0 + 4)))
                        att_started = [False]

                        def emit_att(a, is_last, ps_att=ps_att, pTq=pTq,
                                     att_started=att_started):
                            nc.tensor.matmul(
                                ps_att, lhsT=v_t[a],
                                rhs=pTq[a // 4][:, a % 4, :],
                                start=(not att_started[0]), stop=is_last)
                            att_started[0] = True

                        def drain_atts(si_last, limit=99):
                            n = 0
                            while pending and n < limit:
                                a = pending[0]
                                if (a | 1) > si_last - 1:
                                    break
                                pending.popleft()
                                emit_att(a, False)
                                n += 1

                        def emit_score(si):
                            diag = (si // 4 == tch)
                            nc.tensor.matmul(
                                psAB[(si // 2) % 2][:, si % 2, :],
                                lhsT=kT_c[si // 4][
                                    :, (si % 4) * 128:(si % 4 + 1) * 128],
                                rhs=qT_h[h][:, tsl],
                                start=True, stop=True)
                            if si % 2 == 1:
                                p2 = si // 2
                                k = si // 4
                                qi = k
                                bias = 1.0 if k < tch else 0.0
                                nc.scalar.activation(
                                    pTq[qi][:, 2 * (p2 % 2):2 * (p2 % 2) + 2, :],
                                    psAB[p2 % 2], AF.Exp, bias=bias)
                                if diag:
                                    # lower-triangle e^1 factor on this pair
                                    j2 = 2 * (p2 % 2)
                                    nc.vector.tensor_mul(
                                        pTq[qi][:, j2:j2 + 2, :],
                                        pTq[qi][:, j2:j2 + 2, :],
                                        emask_sb[:, j2:j2 + 2, :])
                                # tree adds: even s8 levels + l4[0] on Pool
                                eng = (nc.gpsimd if p2 % 2 == 0
                                       else nc.vector)
                                eng.tensor_add(
                                    s8[:, p2, :],
                                    pTq[qi][:, 2 * (p2 % 2), :],
                                    pTq[qi][:, 2 * (p2 % 2) + 1, :])
                                if si % 4 == 3:
                                    eng2 = nc.gpsimd if k == 0 else nc.vector
                                    eng2.tensor_add(
                                        l4[:, k, :],
                                        s8[:, 2 * k, :], s8[:, 2 * k + 1, :])
                                if si % 8 == 7:
                                    q8 = si // 8
                                    nc.vector.tensor_add(
                                        l2[:, q8, :],
                                        l4[:, 2 * q8, :], l4[:, 2 * q8 + 1, :])
                                if si == 15:
                                    nc.vector.tensor_add(
                                        accv, l2[:, 0, :], l2[:, 1, :])

                        # 4 bursts; scores mid-slot so exps start early;
                        # atts lag one burst; o_proj 6 DR matmuls/burst
                        for k in range(4):
                            uk = u + k // 2 if u >= 0 else -1
                            jk = (6 * k) % PO_NJ
                            if k == 0:
                                # drain ready work first: the first scores
                                # wait on psAB WAR behind the ACT queue
                                if carry is not None:
                                    cpending, cemit = carry
                                    while cpending:
                                        a = cpending.popleft()
                                        cemit(a, not cpending)
                                    carry = None
                            if uk >= 0:
                                po_step(pop, jk, uk, po_state)
                                po_step(pop, jk + 1, uk, po_state)
                            drain_atts(4 * k, limit=2)
                            emit_score(4 * k)
                            emit_score(4 * k + 1)
                            if k == 0 and prev is not None:
                                emit_denom_flush(prev)
                                prev = None
                            if uk >= 0:
                                po_step(pop, jk + 2, uk, po_state)
                                po_step(pop, jk + 3, uk, po_state)
                            drain_atts(4 * k, limit=2)
                            emit_score(4 * k + 2)
                            emit_score(4 * k + 3)
                            if uk >= 0:
                                po_step(pop, jk + 4, uk, po_state)
                                po_step(pop, jk + 5, uk, po_state)
                            drain_atts(4 * k)
                        carry = (pending, emit_att)
                        prev = (accv, ps_att, tch, h)

                    # last window's att drain + softmax flush
                    cpending, cemit = carry
                    while cpending:
                        a = cpending.popleft()
                        cemit(a, not cpending)
                    emit_denom_flush(prev)

                # ------------- phase 3 tail: remaining o_proj units ----------
                with tc.tile_pool(name="pot", bufs=4, space="PSUM") as pot:
                    # hp0 halves first: hp1 stalls on the final head flush
                    po_state = [None]
                    po_state2 = [None]
                    for u in range(2 * (NW - PO_LAG), 2 * NW):
                        for j in (0, 1, 2, 6, 7, 8, 3, 4, 5, 9, 10, 11):
                            st = po_state if j < 6 else po_state2
                            po_step(pot, j, u, st)
    nc.finalize()
    return nc


def _get_program():
    global _PROGRAM
    if _PROGRAM is None:
        _PROGRAM = _build_program()
    return _PROGRAM


def _rope_cos_sin():
    inv_freq = 1.0 / (10000.0 ** (np.arange(0, D, 2, dtype=np.float64) / D))
    t = np.arange(ROPE_MAX, dtype=np.float64)
    freqs = np.outer(t, inv_freq)                       # [S, D/2]
    emb = np.concatenate([freqs, freqs], axis=-1)       # [S, D]
    return np.cos(emb).astype(np.float32), np.sin(emb).astype(np.float32)


def _fold_rope(w, b, nheads, scale):
    """Fold per-head RoPE (position index = head index) into weight rows.

    w: [nheads*D, C], b: [nheads*D]. Returns rotated (and scaled) copies.
    rope(q)[i] = q[i]*cos[i] + rot_half(q)[i]*sin[i],
    rot_half(q)[i] = -q[i+64] (i<64) else q[i-64].
    """
    cos, sin = _rope_cos_sin()
    w = w.reshape(nheads, D, -1)
    b = b.reshape(nheads, D)
    c = cos[:nheads][:, :, None]          # [nheads, D, 1]
    s = sin[:nheads][:, :, None]
    w_rot = np.empty_like(w)
    hD = D // 2
    w_rot[:, :hD] = w[:, :hD] * c[:, :hD] - w[:, hD:] * s[:, :hD]
    w_rot[:, hD:] = w[:, hD:] * c[:, hD:] + w[:, :hD] * s[:, hD:]
    cb = cos[:nheads]
    sb = sin[:nheads]
    b_rot = np.empty_like(b)
    b_rot[:, :hD] = b[:, :hD] * cb[:, :hD] - b[:, hD:] * sb[:, :hD]
    b_rot[:, hD:] = b[:, hD:] * cb[:, hD:] + b[:, :hD] * sb[:, hD:]
    return (w_rot.reshape(nheads * D, -1) * scale).astype(np.float32), \
           (b_rot.reshape(nheads * D) * scale).astype(np.float32)


BF = ml_dtypes.bfloat16
F8 = ml_dtypes.float8_e4m3fn


def _split8(a, prescale):
    """Dekker split of (a*prescale) into fp8 hi+lo, laid out as
    [partition p, 4*P + 2*hl + ktile, m] for wide 3-dim DMAs.

    a: [DIM, m] (contraction-major)."""
    s = (a.astype(np.float32) * np.float32(prescale))
    hi = s.astype(F8)
    lo = (s - hi.astype(np.float32)).astype(F8)
    hl = np.stack([hi, lo], axis=0)          # [2, DIM, m]
    m = a.shape[1]
    hl = hl.reshape(2, 8, 2, 128, m)         # [hl, P, i, p, m]
    hl = hl.transpose(3, 1, 0, 2, 4)         # [p, P, hl, i, m]
    return np.ascontiguousarray(hl.reshape(128, 32, m))


def _host_inputs(x, wq, bq, wk, bk, wv, bv, wo, bo):
    """Build the per-core input maps."""
    scale = float(D) ** -0.5
    wq_r, bq_r = _fold_rope(wq.astype(np.float32), bq.astype(np.float32), H, scale)
    wk_r, bk_r = _fold_rope(wk.astype(np.float32), bk.astype(np.float32), KV, 1.0)

    # diagonal tile multiplier: e^mask, mask[p, j, t'] = 1 if j*128 + p <= t'
    p_idx = np.arange(128)[:, None, None]
    j_idx = np.arange(4)[None, :, None]
    t_idx = np.arange(512)[None, None, :]
    emask = np.where((j_idx * 128 + p_idx) <= t_idx,
                     np.float32(np.e), np.float32(1.0)).astype(BF)

    x8 = [_split8(x[b].T, XS) for b in range(B)]
    wq8 = {}
    wk8 = {}
    wv8 = {}
    wo8 = {}
    for g in range(G):
        qs = slice(512 * g, 512 * (g + 1))
        ks = slice(128 * g, 128 * (g + 1))
        wq8[g] = _split8(wq_r[qs].T, WSQ)
        wk8[g] = _split8(wk_r[ks].T, WSK)
        wv8[g] = _split8(wv[ks].astype(np.float32).T, WSV)
        # woT [512 c, 2048 oc] -> [hp, p, 2*hl+i, oc], c = (2*hp+i)*128+p
        woT = wo[:, qs].astype(np.float32).T
        s = woT * np.float32(WSO)
        hi = s.astype(F8)
        lo = (s - hi.astype(np.float32)).astype(F8)
        hl = np.stack([hi, lo], axis=0)          # [hl, 512, oc]
        hl = hl.reshape(2, 2, 2, 128, DIM)       # [hl, hp, i, p, oc]
        hl = hl.transpose(1, 3, 0, 2, 4)         # [hp, p, hl, i, oc]
        wo8[g] = np.ascontiguousarray(hl.reshape(2, 128, 4, DIM))

    in_maps = []
    for core in range(NCORES):
        b, g = divmod(core, G)
        qs = slice(512 * g, 512 * (g + 1))
        ks = slice(128 * g, 128 * (g + 1))
        in_maps.append({
            "x8": x8[b],
            "wq8": wq8[g],
            "wk8": wk8[g],
            "wv8": wv8[g],
            "wo8": wo8[g],
            "bq": np.ascontiguousarray(bq_r[qs].reshape(4, 128).T),
            "bk": np.ascontiguousarray(bk_r[ks].reshape(128, 1)),
            "bv_col": np.ascontiguousarray(
                bv[ks].astype(np.float32).reshape(128, 1)),
            "ident": np.eye(128, dtype=np.float32).astype(BF),
            "emask": emask,
            # 1/16: ps_sum = D/16 so rinv = 16/D (attP = fp8(16*attT_norm))
            "ones": np.full((128, 128), 1.0 / 16.0, dtype=np.float32),
        })
    return in_maps


def run_cores(inputs, trace=False, **kw):
    nc = _get_program()
    in_maps = _host_inputs(**inputs)
    res = run_bass_kernel_spmd(nc, in_maps, list(range(NCORES)), trace=trace, **kw)
    return res


def assemble(results, bo):
    bo = bo.astype(np.float32)
    out = np.empty((B, T, DIM), dtype=np.float32)
    for b in range(B):
        acc = results[b * G + 0]["o_part"].astype(np.float32)
        for g in range(1, G):
            acc = acc + results[b * G + g]["o_part"].astype(np.float32)
        out[b] = acc + bo
    return out


def kernel(**inputs) -> np.ndarray:
    res = run_cores(inputs)
    return assemble(res.results, inputs["bo"])

